# revision 1
# baseline (speedup 1.0000x reference)
"""3-layer GCN (message passing) on 8 NeuronCores via Bass/Tile.

Strategy (vertex-cut / dst-sharding, bf16 data path):
  - Nodes are LPT-packed into (core, block) bins by in-degree so every
    128-dst block needs the same ceil(1404/128)=11 edge chunks on every
    core (SPMD program, minimal gather launches). Output rows are
    un-permuted on the host at the end.
  - Layer 1 is re-associated: relu(A_hat (X W1) + b1) = relu((A_hat X') W1
    + b1) with X' = dinv*X pre-scaled on host and expanded to edge order
    (xe), so L1 needs no indirect gathers at all - pure streaming.
  - Layers 2/3: each core computes its Y = dinv*(h @ W) shard; the table
    is AllGathered in FOUR row-pieces interleaved with the producing loop
    (gather indices are host-remapped to the piece-concatenated physical
    layout), then per-chunk indirect gathers fetch 512B bf16 rows.
    GpSimd SWDGE launch issue (~1.45us per 128-row gather) is the
    critical resource; everything else hides under it. The self-loop
    chunk of each block reads the core's OWN ybin shard rows with one
    cheap direct HWDGE DMA (core-independent address), saving one
    indirect launch per block per layer.
  - Scatter-add realized on TensorE with host-built bf16 one-hot scatter
    matrices S[e, dst] = dinv[dst] streamed per block (sall).
  - L1/L2 scatter runs transposed (psT = G^T @ S) so the ReLU epilogue
    writes h^T directly into the resident xts tile (no transposes);
    bias+relu fused into one ScalarE activation per feature half. The
    next layer's phase-1 window is interleaved after each block.
  - L3 scatter runs direct (ps = S^T @ G) to emit [dst, feat] fp32 rows.
"""

import os
import sys

sys.path.insert(0, "/opt/trn_rl_repo")

import numpy as np
import ml_dtypes

BF16 = ml_dtypes.bfloat16

N = 50000
E = 500000
NC = 8
SH = N // NC            # 6250 nodes per core
P = 128
DIN = 128
DH = 256
NBLK = (SH + P - 1) // P      # 49 dst blocks per core
LASTM = SH - (NBLK - 1) * P   # 106 dsts in the last block
AG_WB = [0, 13, 25, 37, NBLK]          # phase1 window boundaries per AG piece
AG_R = [(AG_WB[q] * P, min(AG_WB[q + 1] * P, SH)) for q in range(4)]


def _balance(deg):
    """LPT-pack nodes into (core, block) bins by in-edge weight so every
    block's edge count is ~equal across cores. Returns perm: node -> device
    row (core*SH + block*128 + slot)."""
    import heapq
    w = deg.astype(np.int64)           # regular in-edges per node
    nodes = np.argsort(-w, kind="stable")
    caps = []
    for c in range(NC):
        for b in range(NBLK):
            cap = LASTM if b == NBLK - 1 else P
            caps.append((c * SH + b * P, cap))
    heap = [(0, i) for i in range(len(caps))]
    heapq.heapify(heap)
    fill = [0] * len(caps)
    perm = np.empty(N, dtype=np.int64)
    for n in nodes:
        while True:
            wt, i = heapq.heappop(heap)
            if fill[i] < caps[i][1]:
                break
        perm[n] = caps[i][0] + fill[i]
        fill[i] += 1
        if fill[i] < caps[i][1]:
            heapq.heappush(heap, (wt + int(w[n]), i))
    return perm


def _preprocess(x, edge_index):
    """Host-side graph partitioning. Returns per-core tensors + layout meta."""
    src = np.asarray(edge_index[0], dtype=np.int64)
    dst = np.asarray(edge_index[1], dtype=np.int64)
    deg = np.bincount(dst, minlength=N).astype(np.float64)
    dinv0 = (1.0 / np.sqrt(deg + 1.0)).astype(np.float32)

    perm = _balance(deg)
    inv = np.empty(N, dtype=np.int64)
    inv[perm] = np.arange(N)           # device row -> original node

    # remap everything into device order
    src = perm[src]
    dst = perm[dst]
    x = np.asarray(x, np.float32)[inv]
    dinv = dinv0[inv]

    order = np.argsort(dst, kind="stable")
    s_s = src[order]
    d_s = dst[order]
    bounds = np.searchsorted(d_s, np.arange(NC + 1) * SH)

    # chunk counts per (core, block); cpb shared across cores (SPMD program)
    cnts = np.zeros((NC, NBLK), dtype=np.int64)
    for c in range(NC):
        lo, hi = bounds[c], bounds[c + 1]
        blk = (d_s[lo:hi] - c * SH) >> 7
        cnts[c] = np.bincount(blk, minlength=NBLK)
    cpb = np.maximum(1, (cnts.max(axis=0) + P - 1) // P) + 1  # + self chunk
    cum = np.concatenate([[0], np.cumsum(cpb)])
    nch = int(cum[-1])

    xs = (dinv[:, None] * np.asarray(x, np.float32)).astype(BF16)  # [N, 128]

    per_core = []
    for c in range(NC):
        lo, hi = bounds[c], bounds[c + 1]
        sc = s_s[lo:hi]
        dc = d_s[lo:hi] - c * SH
        blk = dc >> 7
        n_e = hi - lo
        starts = np.concatenate([[0], np.cumsum(cnts[c])])
        pos = np.arange(n_e) - starts[blk]
        col = cum[blk] + (pos >> 7)     # chunk column
        prow = pos & 127                # partition (edge slot)

        idx_all = np.zeros((P, nch), dtype=np.int32)
        # physical table row under 4-piece split-AllGather layout
        gc = sc // SH
        gr = sc % SH
        phys = np.zeros_like(sc)
        for q in range(4):
            lo, hi = AG_R[q]
            sel = (gr >= lo) & (gr < hi)
            phys[sel] = NC * lo + gc[sel] * (hi - lo) + (gr[sel] - lo)
        idx_all[prow, col] = phys.astype(np.int32)

        # host-built scatter matrices: s_all[p, col*128 + dblk] = dinv[dst]
        s_all = np.zeros((P, nch, P), dtype=np.float32)
        s_all[prow, col, dc & 127] = dinv[dc + c * SH]

        # L1 expanded edge table: xe[p, col*128 + j] = xs[src, j] (0 pads)
        xe = np.zeros((P, nch, DIN), dtype=BF16)
        xe[prow, col, :] = xs[sc]

        # self chunk (last chunk of each block): diagonal S; own rows in xe
        for b in range(NBLK):
            mm = LASTM if b == NBLK - 1 else P
            selfcol = cum[b + 1] - 1
            own = c * SH + b * P + np.arange(mm)
            s_all[np.arange(mm), selfcol, np.arange(mm)] = dinv[own]
            xe[:mm, selfcol, :] = xs[own]
        s_all = s_all.reshape(P, nch * P).astype(BF16)
        xe = xe.reshape(P, nch * DIN)

        # dinv of own shard in [p, w] window layout
        ids = c * SH + np.arange(NBLK * P)
        valid = ids < (c + 1) * SH
        dc_own = np.where(valid, dinv[np.minimum(ids, N - 1)], 0.0)
        dc_own = dc_own.reshape(NBLK, P).T.astype(np.float32).copy()

        per_core.append({
            "idx": idx_all,
            "sall": s_all,
            "xe": xe,
            "dco": dc_own,
        })

    meta = {"cpb": cpb.tolist(), "cum": cum.tolist(), "nch": nch}
    return per_core, meta, perm


def _build_program(meta):
    from concourse import bass, bacc, mybir
    import concourse.tile as tile

    f32 = mybir.dt.float32
    bf16 = mybir.dt.bfloat16
    i32 = mybir.dt.int32
    cpb, cum, nch = meta["cpb"], meta["cum"], meta["nch"]
    mxcp = max(cpb)

    nc = bacc.Bacc("TRN2", target_bir_lowering=False, debug=False,
                   dynamic_dma_scratch_size=65536)

    xe = nc.declare_dram_parameter("xe", [P, nch * DIN], bf16, isOutput=False)
    idx = nc.declare_dram_parameter("idx", [P, nch], i32, isOutput=False)
    sall = nc.declare_dram_parameter("sall", [P, nch * P], bf16, isOutput=False)
    dco = nc.declare_dram_parameter("dco", [P, NBLK], f32, isOutput=False)
    w1 = nc.declare_dram_parameter("w1", [P, DH], bf16, isOutput=False)
    w2p = nc.declare_dram_parameter("w2p", [P, 2 * DH], bf16, isOutput=False)
    w3p = nc.declare_dram_parameter("w3p", [P, 2 * DH], bf16, isOutput=False)
    bt = nc.declare_dram_parameter("bt", [P, 4], f32, isOutput=False)
    bf3 = nc.declare_dram_parameter("bf3", [P, DH], f32, isOutput=False)
    outp = nc.declare_dram_parameter("out", [SH, DH], f32, isOutput=True)

    ybin2 = nc.dram_tensor("ybin2", [SH, DH], bf16)
    ybout2 = nc.dram_tensor("ybout2", [N, DH], bf16, addr_space="Shared")
    ybin3 = nc.dram_tensor("ybin3", [SH, DH], bf16)
    ybout3 = nc.dram_tensor("ybout3", [N, DH], bf16, addr_space="Shared")

    AG = mybir.AluOpType
    ACT = mybir.ActivationFunctionType

    with tile.TileContext(nc, linearize=bool(os.environ.get("KLIN"))) as tc:
        with (
            tc.tile_pool(name="const", bufs=1) as cp_,
            tc.tile_pool(name="sb", bufs=3) as sb,
            tc.tile_pool(name="sp", bufs=4) as sp,
            tc.tile_pool(name="gp", bufs=3) as gp,
            tc.tile_pool(name="xb", bufs=2) as xbp,
            tc.tile_pool(name="pp", bufs=2, space="PSUM") as pp,
            tc.tile_pool(name="ph", bufs=6, space="PSUM") as ph,
        ):
            w1sb = cp_.tile([P, DH], dtype=bf16)
            nc.sync.dma_start(out=w1sb[:], in_=w1[:, :])
            w2sb = cp_.tile([P, 2 * DH], dtype=bf16)
            nc.sync.dma_start(out=w2sb[:], in_=w2p[:, :])
            w3sb = cp_.tile([P, 2 * DH], dtype=bf16)
            nc.sync.dma_start(out=w3sb[:], in_=w3p[:, :])
            btsb = cp_.tile([P, 4], dtype=f32)
            nc.sync.dma_start(out=btsb[:], in_=bt[:, :])
            bf3sb = cp_.tile([P, DH], dtype=f32)
            nc.sync.dma_start(out=bf3sb[:], in_=bf3[:, :])
            idxsb = cp_.tile([P, nch], dtype=i32)
            nc.sync.dma_start(out=idxsb[:], in_=idx[:, :])
            dcosb = cp_.tile([P, NBLK], dtype=f32)
            nc.sync.dma_start(out=dcosb[:], in_=dco[:, :])
            # resident transposed activations h^T: half h at cols [h*SH, ...)
            xts = cp_.tile([P, 2 * SH], dtype=bf16)

            def ld_s(b, cp):
                """Load the block's host-built scatter matrices (bf16)."""
                st = sp.tile([P, mxcp * P], dtype=bf16, tag="st")
                nc.sync.dma_start(
                    out=st[:, :cp * P],
                    in_=sall[:, cum[b] * P:(cum[b] + cp) * P])
                return st

            def phase1_win(wsb, ybin, w):
                """One window of Y = dinv * (h @ W) from xts -> ybin rows."""
                m = LASTM if w == NBLK - 1 else P
                ps = pp.tile([P, DH], dtype=f32, tag="ps")
                for h in range(2):
                    nc.tensor.matmul(
                        out=ps[:m, :],
                        lhsT=xts[:, h * SH + w * P:h * SH + w * P + m],
                        rhs=wsb[:, h * DH:(h + 1) * DH],
                        start=(h == 0), stop=(h == 1))
                ysb = sb.tile([P, DH], dtype=bf16, tag="ysb")
                nc.scalar.activation(out=ysb[:m, :], in_=ps[:m, :],
                                     func=ACT.Copy,
                                     scale=dcosb[:m, w:w + 1])
                nc.sync.dma_start(out=ybin[w * P:w * P + m, :],
                                  in_=ysb[:m, :])

            def all_gather_piece(ybin, ybout, q):
                lo, hi = AG_R[q]
                nc.gpsimd.collective_compute(
                    "AllGather", AG.bypass,
                    replica_groups=[list(range(NC))],
                    ins=[ybin[lo:hi, :].opt()],
                    outs=[ybout[NC * lo:NC * hi, :].opt()])

            # ---------------- Layer 1: streamed edge table ------------------
            for b in range(NBLK):
                cp = cpb[b]
                m = LASTM if b == NBLK - 1 else P
                xet = xbp.tile([P, mxcp * DIN], dtype=bf16, tag="xet")
                nc.sync.dma_start(
                    out=xet[:, :cp * DIN],
                    in_=xe[:, cum[b] * DIN:(cum[b] + cp) * DIN])
                st = ld_s(b, cp)
                psa = ph.tile([P, P], dtype=f32, tag="half")
                for k in range(cp):
                    nc.tensor.matmul(
                        out=psa[:, :m],
                        lhsT=xet[:, k * DIN:(k + 1) * DIN],
                        rhs=st[:, k * P:k * P + m],
                        start=(k == 0), stop=(k == cp - 1))
                agg = sb.tile([P, P], dtype=bf16, tag="agg")
                nc.scalar.activation(out=agg[:, :m], in_=psa[:, :m],
                                     func=ACT.Copy)
                psb = [ph.tile([P, P], dtype=f32, tag="half", name=f"psb{h}")
                       for h in range(2)]
                for h in range(2):
                    nc.tensor.matmul(
                        out=psb[h][:, :m],
                        lhsT=w1sb[:, h * P:(h + 1) * P],
                        rhs=agg[:, :m],
                        start=True, stop=True)
                for h in range(2):
                    nc.scalar.activation(
                        out=xts[:, h * SH + b * P:h * SH + b * P + m],
                        in_=psb[h][:, :m],
                        func=ACT.Relu, bias=btsb[:, h:h + 1])
                phase1_win(w2sb, ybin2, b)
                if b + 1 in AG_WB[1:4]:
                    all_gather_piece(ybin2, ybout2, AG_WB.index(b + 1) - 1)

            def scatter_t(table, ybin_loc, bofs, nwsb, nybin, nybout):
                """Transposed scatter + fused bias/relu epilogue -> xts,
                with the next layer's phase1 window interleaved per block."""
                for b in range(NBLK):
                    cp = cpb[b]
                    m = LASTM if b == NBLK - 1 else P
                    gt = gp.tile([P, mxcp * DH], dtype=bf16, tag="gt")
                    for k in range(cp - 1):
                        nc.gpsimd.indirect_dma_start(
                            out=gt[:, k * DH:(k + 1) * DH], out_offset=None,
                            in_=table[:, :],
                            in_offset=bass.IndirectOffsetOnAxis(
                                ap=idxsb[:, cum[b] + k:cum[b] + k + 1],
                                axis=0))
                    nc.sync.dma_start(
                        out=gt[:m, (cp - 1) * DH:cp * DH],
                        in_=ybin_loc[b * P:b * P + m, :])
                    st = ld_s(b, cp)
                    pst = [ph.tile([P, P], dtype=f32, tag="half", name=f"pst{h}")
                           for h in range(2)]
                    for k in range(cp):
                        for h in range(2):
                            nc.tensor.matmul(
                                out=pst[h][:, :m],
                                lhsT=gt[:, k * DH + h * P:k * DH + (h + 1) * P],
                                rhs=st[:, k * P:k * P + m],
                                start=(k == 0), stop=(k == cp - 1))
                    for h in range(2):
                        nc.scalar.activation(
                            out=xts[:, h * SH + b * P:h * SH + b * P + m],
                            in_=pst[h][:, :m],
                            func=ACT.Relu, bias=btsb[:, bofs + h:bofs + h + 1])
                    phase1_win(nwsb, nybin, b)
                    if b + 1 in AG_WB[1:4]:
                        all_gather_piece(nybin, nybout, AG_WB.index(b + 1) - 1)

            all_gather_piece(ybin2, ybout2, 3)
            scatter_t(ybout2, ybin2, 2, w3sb, ybin3, ybout3)
            all_gather_piece(ybin3, ybout3, 3)

            dbg = os.environ.get("KDBG")
            if dbg:
                # dump a bf16 [SH, DH] DRAM tensor to outp (cast to f32)
                src_t = {"yb2": ybin2, "yb3": ybin3}[dbg]
                for b in range(NBLK):
                    m = LASTM if b == NBLK - 1 else P
                    t = sb.tile([P, DH], dtype=bf16, tag="dbg")
                    nc.sync.dma_start(out=t[:m, :],
                                      in_=src_t[b * P:b * P + m, :])
                    t2 = sb.tile([P, DH], dtype=f32, tag="dbg2")
                    nc.vector.tensor_copy(out=t2[:m, :], in_=t[:m, :])
                    nc.sync.dma_start(out=outp[b * P:b * P + m, :],
                                      in_=t2[:m, :])

            # ---------------- Layer 3 scatter: direct [dst, feat] ----------
            for b in range(NBLK):
                cp = cpb[b]
                m = LASTM if b == NBLK - 1 else P
                gt = gp.tile([P, mxcp * DH], dtype=bf16, tag="gt")
                for k in range(cp - 1):
                    nc.gpsimd.indirect_dma_start(
                        out=gt[:, k * DH:(k + 1) * DH], out_offset=None,
                        in_=ybout3[:, :],
                        in_offset=bass.IndirectOffsetOnAxis(
                            ap=idxsb[:, cum[b] + k:cum[b] + k + 1], axis=0))
                nc.sync.dma_start(
                    out=gt[:m, (cp - 1) * DH:cp * DH],
                    in_=ybin3[b * P:b * P + m, :])
                st = ld_s(b, cp)
                ps3 = pp.tile([P, DH], dtype=f32, tag="ps")
                for k in range(cp):
                    nc.tensor.matmul(
                        out=ps3[:m, :],
                        lhsT=st[:, k * P:k * P + m],
                        rhs=gt[:, k * DH:(k + 1) * DH],
                        start=(k == 0), stop=(k == cp - 1))
                osb = sb.tile([P, DH], dtype=f32, tag="osb")
                nc.vector.tensor_tensor(out=osb[:m, :], in0=ps3[:m, :],
                                        in1=bf3sb[:m, :], op=AG.add)
                nc.sync.dma_start(out=outp[b * P:b * P + m, :],
                                  in_=osb[:m, :])

    nc.compile()
    return nc


def kernel(x, edge_index, W1, b1, W2, b2, W3, b3, _trace=False):
    from concourse.bass_utils import run_bass_kernel_spmd

    x = np.asarray(x, dtype=np.float32)
    per_core, meta, perm = _preprocess(x, edge_index)
    nc = _build_program(meta)

    w2 = np.asarray(W2, np.float32)
    w3 = np.asarray(W3, np.float32)
    w2p = np.concatenate([w2[0:P, :], w2[P:2 * P, :]], axis=1).astype(BF16)
    w3p = np.concatenate([w3[0:P, :], w3[P:2 * P, :]], axis=1).astype(BF16)
    b1v = np.asarray(b1, np.float32)
    b2v = np.asarray(b2, np.float32)
    bt = np.stack([b1v[0:P], b1v[P:2 * P], b2v[0:P], b2v[P:2 * P]],
                  axis=1).astype(np.float32)
    common = {
        "w1": np.asarray(W1, np.float32).astype(BF16),
        "w2p": w2p,
        "w3p": w3p,
        "bt": bt,
        "bf3": np.broadcast_to(np.asarray(b3, np.float32), (P, DH)).copy(),
    }
    in_maps = []
    for c in range(NC):
        m = dict(common)
        m.update(per_core[c])
        m["dco"] = per_core[c]["dco"]
        in_maps.append(m)

    res = run_bass_kernel_spmd(nc, in_maps, list(range(NC)), trace=_trace)
    shards = [res.results[c]["out"] for c in range(NC)]
    out = np.concatenate(shards, axis=0)[perm]
    if _trace:
        return out, res
    return out



# revision 3
# speedup vs baseline: 1.0618x; 1.0618x over previous
"""3-layer GCN (message passing) on 8 NeuronCores via Bass/Tile.

Strategy (vertex-cut / dst-sharding, bf16 data path):
  - Nodes are LPT-packed into (core, block) bins by in-degree so every
    128-dst block needs a similar number of edge chunks on every core
    (SPMD program). Output rows are un-permuted on the host at the end.
  - Self loops are regular edges (src=dst), no special casing.
  - Layer 1 is re-associated: relu(A_hat (X W1) + b1) = relu((A_hat X') W1
    + b1) with X' = dinv*X pre-scaled on host and expanded to edge order
    (xe), so L1 needs no indirect gathers at all - pure streaming.
  - Layers 2/3: each core computes its Y = dinv*(h @ W) shard; the table
    is AllGathered in FOUR row-pieces (separate DRAM tensors so the tile
    dep tracker gives piece-granular ordering), interleaved with the
    producing loop. Edges are grouped by (7-block group, source piece)
    and fetched with ONE batched gpsimd dma_gather per (group, piece)
    (SWDGE cost ~1us fixed + 0.34ns/row, so batching ~3k rows per launch
    amortizes the launch cost ~25x vs per-chunk indirect DMA).
  - Scatter-add realized on TensorE with one-hot scatter matrices
    S[e, dst] = dinv[dst] built ON-CHIP per 128-col chunk by a single
    fused VectorE tensor_scalar: (iota == dstcol[:,k]) * dinvd[:,k].
  - L1/L2 scatter runs transposed (psT = G^T @ S) so the ReLU epilogue
    writes h^T directly into the resident xts tile; bias+relu fused into
    one ScalarE activation per feature half. The next layer's phase-1
    window is interleaved after each block.
  - L3 scatter runs direct (ps = S^T @ G) to emit [dst, feat] fp32 rows.
"""

import os
import sys

sys.path.insert(0, "/opt/trn_rl_repo")

import numpy as np
import ml_dtypes

BF16 = ml_dtypes.bfloat16

N = 50000
E = 500000
NC = 8
SH = N // NC            # 6250 nodes per core
P = 128
DIN = 128
DH = 256
NBLK = (SH + P - 1) // P      # 49 dst blocks per core
LASTM = SH - (NBLK - 1) * P   # 106 dsts in the last block
GSZ = 7                       # blocks per gather group
NG = NBLK // GSZ              # 7 groups
NQ = 4                        # AllGather pieces
AG_WB = [0, 13, 25, 37, NBLK]            # window boundaries per AG piece
AG_LO = [w * P for w in AG_WB[:4]]       # piece start rows (per core)
RQ = [min(AG_WB[q + 1] * P, SH) - AG_LO[q] for q in range(NQ)]  # piece rows


def _balance(deg):
    """LPT-pack nodes into (core, block) bins by in-edge weight so every
    block's edge count is ~equal across cores. Returns perm: node -> device
    row (core*SH + block*128 + slot)."""
    import heapq
    w = deg.astype(np.int64)
    nodes = np.argsort(-w, kind="stable")
    caps = []
    for c in range(NC):
        for b in range(NBLK):
            cap = LASTM if b == NBLK - 1 else P
            caps.append((c * SH + b * P, cap))
    heap = [(0, i) for i in range(len(caps))]
    heapq.heapify(heap)
    fill = [0] * len(caps)
    perm = np.empty(N, dtype=np.int64)
    for n in nodes:
        while True:
            wt, i = heapq.heappop(heap)
            if fill[i] < caps[i][1]:
                break
        perm[n] = caps[i][0] + fill[i]
        fill[i] += 1
        if fill[i] < caps[i][1]:
            heapq.heappush(heap, (wt + int(w[n]), i))
    return perm


def _layout():
    """Static (data-independent) meta helpers."""
    pass


def _preprocess(x, edge_index):
    """Host-side graph partitioning. Returns per-core tensors + layout meta."""
    src = np.asarray(edge_index[0], dtype=np.int64)
    dst = np.asarray(edge_index[1], dtype=np.int64)
    deg = np.bincount(dst, minlength=N).astype(np.float64)
    dinv0 = (1.0 / np.sqrt(deg + 1.0)).astype(np.float32)

    perm = _balance(deg)
    inv = np.empty(N, dtype=np.int64)
    inv[perm] = np.arange(N)           # device row -> original node
    src = perm[src]
    dst = perm[dst]
    x = np.asarray(x, np.float32)[inv]
    dinv = dinv0[inv]

    # self loops as regular edges
    loop = np.arange(N, dtype=np.int64)
    src = np.concatenate([src, loop])
    dst = np.concatenate([dst, loop])

    order = np.argsort(dst, kind="stable")
    s_s = src[order]
    d_s = dst[order]
    bounds = np.searchsorted(d_s, np.arange(NC + 1) * SH)

    xs = (dinv[:, None] * x).astype(BF16)  # [N, 128] pre-scaled features

    # source piece + piece-local table row for every edge
    gc = s_s // SH
    gr = s_s % SH
    q_of = np.searchsorted(np.array(AG_LO[1:]), gr, side="right")  # [TE]
    rq = np.array(RQ)[q_of]
    lo = np.array(AG_LO)[q_of]
    prow = gc * rq + (gr - lo)         # row within piece-q gathered table
    blk = np.minimum((d_s % SH) >> 7, NBLK - 1)

    # per-core per-(block, piece) counts -> shared chunk counts k2
    cnt = np.zeros((NC, NBLK, NQ), dtype=np.int64)
    for c in range(NC):
        lo_, hi_ = bounds[c], bounds[c + 1]
        np.add.at(cnt[c], (blk[lo_:hi_], q_of[lo_:hi_]), 1)
    k2 = (cnt.max(axis=0) + P - 1) // P            # [NBLK, NQ]
    cnt1 = cnt.sum(axis=2)                         # [NC, NBLK]
    k1 = np.maximum(1, (cnt1.max(axis=0) + P - 1) // P)  # [NBLK]
    cum1 = np.concatenate([[0], np.cumsum(k1)]).astype(int)
    nch1 = int(cum1[-1])

    # L2/L3 column layout: group-major, then piece, then block
    colstart = np.zeros((NBLK, NQ), dtype=np.int64)
    cstart = np.zeros((NG, NQ), dtype=np.int64)
    K = np.zeros((NG, NQ), dtype=np.int64)
    gstart = np.zeros(NG + 1, dtype=np.int64)
    pos = 0
    for g in range(NG):
        gstart[g] = pos
        for q in range(NQ):
            cstart[g, q] = pos
            for b in range(g * GSZ, (g + 1) * GSZ):
                colstart[b, q] = pos
                pos += k2[b, q]
            K[g, q] = pos - cstart[g, q]
    gstart[NG] = pos
    nch2 = int(pos)
    cols2 = [
        [int(colstart[b, q]) + i for q in range(NQ) for i in range(k2[b, q])]
        for b in range(NBLK)
    ]
    wstart = np.zeros((NG, NQ), dtype=np.int64)
    tw = 0
    for g in range(NG):
        for q in range(NQ):
            wstart[g, q] = tw
            tw += 8 * int(K[g, q])
    TW = int(tw)

    per_core = []
    for c in range(NC):
        lo_, hi_ = bounds[c], bounds[c + 1]
        b_c = blk[lo_:hi_]
        q_c = q_of[lo_:hi_]
        pr_c = prow[lo_:hi_]
        d_c = d_s[lo_:hi_] - c * SH
        s_c = s_s[lo_:hi_]

        # ---- L2/L3 slot assignment: order by (block, piece) ----
        key = b_c * NQ + q_c
        o2 = np.argsort(key, kind="stable")
        key_s = key[o2]
        # position within each (b,q) run
        starts = np.searchsorted(key_s, np.arange(NBLK * NQ))
        pos_in = np.arange(len(key_s)) - starts[key_s]
        chunk = pos_in >> 7
        part = pos_in & 127
        col = colstart.reshape(-1)[key_s] + chunk     # global chunk col

        dstcol2 = np.full((P, nch2), -1.0, dtype=np.float32)
        dinvd2 = np.zeros((P, nch2), dtype=np.float32)
        dloc = d_c[o2]
        dstcol2[part, col] = (dloc & 127).astype(np.float32)
        dinvd2[part, col] = dinv[dloc + c * SH]

        big = np.zeros(P * nch2, dtype=np.int16)
        big[col * P + part] = pr_c[o2].astype(np.int16)
        idx2 = np.zeros((P, TW), dtype=np.int16)
        for g in range(NG):
            for q in range(NQ):
                kk = int(K[g, q])
                if kk == 0:
                    continue
                arr = big[int(cstart[g, q]) * P:(int(cstart[g, q]) + kk) * P]
                wr = arr.reshape(-1, 16).T            # [16, 8*kk]
                ws = int(wstart[g, q])
                idx2[:, ws:ws + 8 * kk] = np.tile(wr, (8, 1))

        # ---- L1 slot assignment: order by block only ----
        o1 = np.argsort(b_c, kind="stable")
        b1s = b_c[o1]
        starts1 = np.searchsorted(b1s, np.arange(NBLK))
        pos1 = np.arange(len(b1s)) - starts1[b1s]
        chunk_1 = pos1 >> 7
        part1 = pos1 & 127
        col1 = cum1[b1s] + chunk_1

        xe = np.zeros((P, nch1, DIN), dtype=BF16)
        xe[part1, col1, :] = xs[s_c[o1]]
        dstcol1 = np.full((P, nch1), -1.0, dtype=np.float32)
        dinvd1 = np.zeros((P, nch1), dtype=np.float32)
        d1 = d_c[o1]
        dstcol1[part1, col1] = (d1 & 127).astype(np.float32)
        dinvd1[part1, col1] = dinv[d1 + c * SH]
        xe = xe.reshape(P, nch1 * DIN)

        # dinv of own shard in [p, w] window layout (phase-1 scaling)
        ids = c * SH + np.arange(NBLK * P)
        valid = ids < (c + 1) * SH
        dc_own = np.where(valid, dinv[np.minimum(ids, N - 1)], 0.0)
        dc_own = dc_own.reshape(NBLK, P).T.astype(np.float32).copy()

        per_core.append({
            "xe": xe,
            "dc1": dstcol1, "dv1": dinvd1,
            "dc2": dstcol2, "dv2": dinvd2,
            "idx2": idx2,
            "dco": dc_own,
        })

    meta = {
        "k2": k2.tolist(), "k1": k1.tolist(), "cum1": cum1.tolist(),
        "nch1": nch1, "nch2": nch2, "TW": TW,
        "cstart": cstart.tolist(), "K": K.tolist(),
        "gstart": gstart.tolist(), "wstart": wstart.tolist(),
        "cols2": cols2,
    }
    return per_core, meta, perm


def _build_program(meta):
    from concourse import bass, bacc, mybir
    import concourse.tile as tile

    f32 = mybir.dt.float32
    bf16 = mybir.dt.bfloat16
    i16 = mybir.dt.int16
    k1 = meta["k1"]
    cum1 = meta["cum1"]
    nch1 = meta["nch1"]
    nch2 = meta["nch2"]
    TW = meta["TW"]
    cstart = meta["cstart"]
    K = meta["K"]
    gstart = meta["gstart"]
    wstart = meta["wstart"]
    cols2 = meta["cols2"]
    mxk1 = max(k1)
    mxgw = max(gstart[g + 1] - gstart[g] for g in range(NG))

    nc = bacc.Bacc("TRN2", target_bir_lowering=False, debug=False,
                   dynamic_dma_scratch_size=65536)

    xe = nc.declare_dram_parameter("xe", [P, nch1 * DIN], bf16, isOutput=False)
    dc1 = nc.declare_dram_parameter("dc1", [P, nch1], f32, isOutput=False)
    dv1 = nc.declare_dram_parameter("dv1", [P, nch1], f32, isOutput=False)
    dc2 = nc.declare_dram_parameter("dc2", [P, nch2], f32, isOutput=False)
    dv2 = nc.declare_dram_parameter("dv2", [P, nch2], f32, isOutput=False)
    idx2 = nc.declare_dram_parameter("idx2", [P, TW], i16, isOutput=False)
    dco = nc.declare_dram_parameter("dco", [P, NBLK], f32, isOutput=False)
    w1 = nc.declare_dram_parameter("w1", [P, DH], bf16, isOutput=False)
    w2p = nc.declare_dram_parameter("w2p", [P, 2 * DH], bf16, isOutput=False)
    w3p = nc.declare_dram_parameter("w3p", [P, 2 * DH], bf16, isOutput=False)
    bt = nc.declare_dram_parameter("bt", [P, 4], f32, isOutput=False)
    bf3 = nc.declare_dram_parameter("bf3", [P, DH], f32, isOutput=False)
    iot = nc.declare_dram_parameter("iot", [P, P], i16, isOutput=False)
    outp = nc.declare_dram_parameter("out", [SH, DH], f32, isOutput=True)

    ybin2 = [nc.dram_tensor(f"ybin2_{q}", [RQ[q], DH], bf16) for q in range(NQ)]
    ybout2 = [nc.dram_tensor(f"ybout2_{q}", [NC * RQ[q], DH], bf16,
                             addr_space="Shared") for q in range(NQ)]
    ybin3 = [nc.dram_tensor(f"ybin3_{q}", [RQ[q], DH], bf16) for q in range(NQ)]
    ybout3 = [nc.dram_tensor(f"ybout3_{q}", [NC * RQ[q], DH], bf16,
                             addr_space="Shared") for q in range(NQ)]

    AG = mybir.AluOpType
    ACT = mybir.ActivationFunctionType

    def piece_of_window(w):
        for q in range(NQ):
            if AG_WB[q] <= w < AG_WB[q + 1]:
                return q
        raise AssertionError

    with tile.TileContext(nc, linearize=bool(os.environ.get("KLIN"))) as tc:
        with (
            tc.tile_pool(name="const", bufs=1) as cp_,
            tc.tile_pool(name="sb", bufs=3) as sb,
            tc.tile_pool(name="stp", bufs=4) as stp,
            tc.tile_pool(name="gp", bufs=2) as gp,
            tc.tile_pool(name="xb", bufs=2) as xbp,
            tc.tile_pool(name="pp", bufs=2, space="PSUM") as pp,
            tc.tile_pool(name="ph", bufs=6, space="PSUM") as ph,
        ):
            w1sb = cp_.tile([P, DH], dtype=bf16)
            nc.sync.dma_start(out=w1sb[:], in_=w1[:, :])
            w2sb = cp_.tile([P, 2 * DH], dtype=bf16)
            nc.sync.dma_start(out=w2sb[:], in_=w2p[:, :])
            w3sb = cp_.tile([P, 2 * DH], dtype=bf16)
            nc.sync.dma_start(out=w3sb[:], in_=w3p[:, :])
            btsb = cp_.tile([P, 4], dtype=f32)
            nc.sync.dma_start(out=btsb[:], in_=bt[:, :])
            bf3sb = cp_.tile([P, DH], dtype=f32)
            nc.sync.dma_start(out=bf3sb[:], in_=bf3[:, :])
            dcosb = cp_.tile([P, NBLK], dtype=f32)
            nc.sync.dma_start(out=dcosb[:], in_=dco[:, :])
            iotsb = cp_.tile([P, P], dtype=i16)
            nc.sync.dma_start(out=iotsb[:], in_=iot[:, :])
            dc1sb = cp_.tile([P, nch1], dtype=f32)
            nc.sync.dma_start(out=dc1sb[:], in_=dc1[:, :])
            dv1sb = cp_.tile([P, nch1], dtype=f32)
            nc.sync.dma_start(out=dv1sb[:], in_=dv1[:, :])
            dc2sb = cp_.tile([P, nch2], dtype=f32)
            nc.sync.dma_start(out=dc2sb[:], in_=dc2[:, :])
            dv2sb = cp_.tile([P, nch2], dtype=f32)
            nc.sync.dma_start(out=dv2sb[:], in_=dv2[:, :])
            idxsb = cp_.tile([P, TW], dtype=i16)
            nc.sync.dma_start(out=idxsb[:], in_=idx2[:, :])
            # resident transposed activations h^T: half h at cols [h*SH, ...)
            xts = cp_.tile([P, 2 * SH], dtype=bf16)

            def sbuild(dcsb, dvsb, col, m):
                """One-hot scatter chunk S[p, c] = (c == dstcol[p]) * dinv."""
                st = stp.tile([P, P], dtype=bf16, tag="st")
                nc.vector.tensor_scalar(
                    out=st[:, :m], in0=iotsb[:, :m],
                    scalar1=dcsb[:, col:col + 1],
                    scalar2=dvsb[:, col:col + 1],
                    op0=AG.is_equal, op1=AG.mult)
                return st

            def phase1_win(wsb, ybinq, w):
                """One window of Y = dinv * (h @ W) from xts -> ybin rows."""
                m = LASTM if w == NBLK - 1 else P
                ps = pp.tile([P, DH], dtype=f32, tag="ps")
                for h in range(2):
                    nc.tensor.matmul(
                        out=ps[:m, :],
                        lhsT=xts[:, h * SH + w * P:h * SH + w * P + m],
                        rhs=wsb[:, h * DH:(h + 1) * DH],
                        start=(h == 0), stop=(h == 1))
                ysb = sb.tile([P, DH], dtype=bf16, tag="ysb")
                nc.scalar.activation(out=ysb[:m, :], in_=ps[:m, :],
                                     func=ACT.Copy,
                                     scale=dcosb[:m, w:w + 1])
                q = piece_of_window(w)
                r0 = w * P - AG_LO[q]
                nc.sync.dma_start(out=ybinq[q][r0:r0 + m, :], in_=ysb[:m, :])

            def all_gather_piece(ybinq, yboutq, q):
                nc.gpsimd.collective_compute(
                    "AllGather", AG.bypass,
                    replica_groups=[list(range(NC))],
                    ins=[ybinq[q][0:RQ[q], :].opt()],
                    outs=[yboutq[q][0:NC * RQ[q], :].opt()])

            gmax = int(os.environ.get("KGMAX", "8"))   # chunks per dma_gather
            sp = not os.environ.get("KMP")             # single_packet flag

            def group_gather(g, tableq):
                """Batched gathers for group g, split into <=gmax-chunk
                dma_gathers per piece (the wrapped-16 idx layout slices
                cleanly at chunk granularity)."""
                gt = gp.tile([P, mxgw * DH], dtype=bf16, tag="gt")
                g0 = gstart[g]
                for q in range(NQ):
                    kk = K[g][q]
                    c0 = cstart[g][q] - g0      # column offset inside tile
                    ws = wstart[g][q]
                    for j0 in range(0, kk, gmax):
                        j1 = min(j0 + gmax, kk)
                        out_ap = gt[:, (c0 + j0) * DH:(c0 + j1) * DH].rearrange(
                            "p (k e) -> p k e", e=DH)
                        nc.gpsimd.dma_gather(
                            out_ap,
                            tableq[q][0:NC * RQ[q], :],
                            idxsb[:, ws + 8 * j0:ws + 8 * j1],
                            P * (j1 - j0),
                            P * (j1 - j0),
                            DH,
                            single_packet=sp)
                return gt

            # ---------------- Layer 1: streamed edge table ------------------
            for b in range(NBLK):
                kb = k1[b]
                m = LASTM if b == NBLK - 1 else P
                xet = xbp.tile([P, mxk1 * DIN], dtype=bf16, tag="xet")
                nc.sync.dma_start(
                    out=xet[:, :kb * DIN],
                    in_=xe[:, cum1[b] * DIN:(cum1[b] + kb) * DIN])
                psa = ph.tile([P, P], dtype=f32, tag="half")
                for i in range(kb):
                    st = sbuild(dc1sb, dv1sb, cum1[b] + i, m)
                    nc.tensor.matmul(
                        out=psa[:, :m],
                        lhsT=xet[:, i * DIN:(i + 1) * DIN],
                        rhs=st[:, :m],
                        start=(i == 0), stop=(i == kb - 1))
                agg = sb.tile([P, P], dtype=bf16, tag="agg")
                nc.scalar.activation(out=agg[:, :m], in_=psa[:, :m],
                                     func=ACT.Copy)
                psb = [ph.tile([P, P], dtype=f32, tag="half", name=f"psb{h}")
                       for h in range(2)]
                for h in range(2):
                    nc.tensor.matmul(
                        out=psb[h][:, :m],
                        lhsT=w1sb[:, h * P:(h + 1) * P],
                        rhs=agg[:, :m],
                        start=True, stop=True)
                for h in range(2):
                    nc.scalar.activation(
                        out=xts[:, h * SH + b * P:h * SH + b * P + m],
                        in_=psb[h][:, :m],
                        func=ACT.Relu, bias=btsb[:, h:h + 1])
                phase1_win(w2sb, ybin2, b)
                if b + 1 in AG_WB[1:4]:
                    all_gather_piece(ybin2, ybout2, AG_WB.index(b + 1) - 1)
            all_gather_piece(ybin2, ybout2, 3)

            # ---------------- Layer 2: transposed scatter -------------------
            for g in range(NG):
                gt = group_gather(g, ybout2)
                g0 = gstart[g]
                for b in range(g * GSZ, (g + 1) * GSZ):
                    m = LASTM if b == NBLK - 1 else P
                    cols = cols2[b]
                    pst = [ph.tile([P, P], dtype=f32, tag="half",
                                   name=f"pst{h}") for h in range(2)]
                    for ci, col in enumerate(cols):
                        lp = col - g0
                        st = sbuild(dc2sb, dv2sb, col, m)
                        for h in range(2):
                            nc.tensor.matmul(
                                out=pst[h][:, :m],
                                lhsT=gt[:, lp * DH + h * P:lp * DH + (h + 1) * P],
                                rhs=st[:, :m],
                                start=(ci == 0), stop=(ci == len(cols) - 1))
                    for h in range(2):
                        nc.scalar.activation(
                            out=xts[:, h * SH + b * P:h * SH + b * P + m],
                            in_=pst[h][:, :m],
                            func=ACT.Relu, bias=btsb[:, 2 + h:2 + h + 1])
                    phase1_win(w3sb, ybin3, b)
                    if b + 1 in AG_WB[1:4]:
                        all_gather_piece(ybin3, ybout3, AG_WB.index(b + 1) - 1)
            all_gather_piece(ybin3, ybout3, 3)

            # ---------------- Layer 3: direct scatter -> out ----------------
            for g in range(NG):
                gt = group_gather(g, ybout3)
                g0 = gstart[g]
                for b in range(g * GSZ, (g + 1) * GSZ):
                    m = LASTM if b == NBLK - 1 else P
                    cols = cols2[b]
                    ps3 = pp.tile([P, DH], dtype=f32, tag="ps")
                    for ci, col in enumerate(cols):
                        lp = col - g0
                        st = sbuild(dc2sb, dv2sb, col, m)
                        nc.tensor.matmul(
                            out=ps3[:m, :],
                            lhsT=st[:, :m],
                            rhs=gt[:, lp * DH:(lp + 1) * DH],
                            start=(ci == 0), stop=(ci == len(cols) - 1))
                    osb = sb.tile([P, DH], dtype=f32, tag="osb")
                    nc.vector.tensor_tensor(out=osb[:m, :], in0=ps3[:m, :],
                                            in1=bf3sb[:m, :], op=AG.add)
                    nc.sync.dma_start(out=outp[b * P:b * P + m, :],
                                      in_=osb[:m, :])

    nc.compile()
    return nc


def kernel(x, edge_index, W1, b1, W2, b2, W3, b3, _trace=False):
    from concourse.bass_utils import run_bass_kernel_spmd

    x = np.asarray(x, dtype=np.float32)
    per_core, meta, perm = _preprocess(x, edge_index)
    nc = _build_program(meta)

    w2 = np.asarray(W2, np.float32)
    w3 = np.asarray(W3, np.float32)
    w2p = np.concatenate([w2[0:P, :], w2[P:2 * P, :]], axis=1).astype(BF16)
    w3p = np.concatenate([w3[0:P, :], w3[P:2 * P, :]], axis=1).astype(BF16)
    b1v = np.asarray(b1, np.float32)
    b2v = np.asarray(b2, np.float32)
    bt = np.stack([b1v[0:P], b1v[P:2 * P], b2v[0:P], b2v[P:2 * P]],
                  axis=1).astype(np.float32)
    common = {
        "w1": np.asarray(W1, np.float32).astype(BF16),
        "w2p": w2p,
        "w3p": w3p,
        "bt": bt,
        "bf3": np.broadcast_to(np.asarray(b3, np.float32), (P, DH)).copy(),
        "iot": np.broadcast_to(np.arange(P, dtype=np.int16), (P, P)).copy(),
    }
    in_maps = []
    for c in range(NC):
        m = dict(common)
        m.update(per_core[c])
        in_maps.append(m)

    res = run_bass_kernel_spmd(nc, in_maps, list(range(NC)), trace=_trace)
    shards = [res.results[c]["out"] for c in range(NC)]
    out = np.concatenate(shards, axis=0)[perm]
    if _trace:
        return out, res
    return out


# revision 26
# speedup vs baseline: 1.0992x; 1.0353x over previous
"""3-layer GCN (message passing) on 8 NeuronCores via Bass/Tile.

Strategy (vertex-cut / dst-sharding, bf16 data path):
  - Nodes are LPT-packed into (core, block) bins by in-degree so every
    128-dst block needs a similar number of edge chunks on every core
    (SPMD program). Output rows are un-permuted on the host at the end.
  - Self loops are regular edges (src=dst), no special casing.
  - Layer 1 is re-associated: relu(A_hat (X W1) + b1) = relu((A_hat X') W1
    + b1) with X' = dinv*X pre-scaled on host and expanded to edge order
    (xe), so L1 needs no indirect gathers at all - pure streaming.
  - Layers 2/3: each core computes its Y = dinv*(h @ W) shard; the table
    is AllGathered in FOUR row-pieces (separate DRAM tensors so the tile
    dep tracker gives piece-granular ordering), interleaved with the
    producing loop. Edges are grouped by (7-block group, source piece)
    and fetched with ONE batched gpsimd dma_gather per (group, piece)
    (SWDGE cost ~1us fixed + 0.34ns/row, so batching ~3k rows per launch
    amortizes the launch cost ~25x vs per-chunk indirect DMA).
  - Scatter-add realized on TensorE with one-hot scatter matrices
    S[e, dst] = dinv[dst] built ON-CHIP per 128-col chunk by a single
    fused VectorE tensor_scalar: (iota == dstcol[:,k]) * dinvd[:,k].
  - L1/L2 scatter runs transposed (psT = G^T @ S) so the ReLU epilogue
    writes h^T directly into the resident xts tile; bias+relu fused into
    one ScalarE activation per feature half. The next layer's phase-1
    window is interleaved after each block.
  - L3 scatter runs direct (ps = S^T @ G) to emit [dst, feat] fp32 rows.
"""

import os
import sys

sys.path.insert(0, "/opt/trn_rl_repo")

import numpy as np
import ml_dtypes

BF16 = ml_dtypes.bfloat16

N = 50000
E = 500000
NC = 8
SH = N // NC            # 6250 nodes per core
P = 128
DIN = 128
DH = 256
NBLK = (SH + P - 1) // P      # 49 dst blocks per core
LASTM = SH - (NBLK - 1) * P   # 106 dsts in the last block
GSZ = 7                       # blocks per gather group
NG = NBLK // GSZ              # 7 groups
NQ = 4                        # AllGather pieces
AG_WB = [0, 13, 25, 37, NBLK]            # window boundaries per AG piece
AG_LO = [w * P for w in AG_WB[:4]]       # piece start rows (per core)
RQ = [min(AG_WB[q + 1] * P, SH) - AG_LO[q] for q in range(NQ)]  # piece rows


def _balance(deg):
    """LPT-pack nodes into (core, block) bins by in-edge weight so every
    block's edge count is ~equal across cores. Returns perm: node -> device
    row (core*SH + block*128 + slot)."""
    import heapq
    w = deg.astype(np.int64)
    nodes = np.argsort(-w, kind="stable")
    caps = []
    for c in range(NC):
        for b in range(NBLK):
            cap = LASTM if b == NBLK - 1 else P
            caps.append((c * SH + b * P, cap))
    heap = [(0, i) for i in range(len(caps))]
    heapq.heapify(heap)
    fill = [0] * len(caps)
    perm = np.empty(N, dtype=np.int64)
    for n in nodes:
        while True:
            wt, i = heapq.heappop(heap)
            if fill[i] < caps[i][1]:
                break
        perm[n] = caps[i][0] + fill[i]
        fill[i] += 1
        if fill[i] < caps[i][1]:
            heapq.heappush(heap, (wt + int(w[n]), i))
    return perm


def _layout():
    """Static (data-independent) meta helpers."""
    pass


def _preprocess(x, edge_index):
    """Host-side graph partitioning. Returns per-core tensors + layout meta."""
    src = np.asarray(edge_index[0], dtype=np.int64)
    dst = np.asarray(edge_index[1], dtype=np.int64)
    deg = np.bincount(dst, minlength=N).astype(np.float64)
    dinv0 = (1.0 / np.sqrt(deg + 1.0)).astype(np.float32)

    perm = _balance(deg)
    inv = np.empty(N, dtype=np.int64)
    inv[perm] = np.arange(N)           # device row -> original node
    src = perm[src]
    dst = perm[dst]
    x = np.asarray(x, np.float32)[inv]
    dinv = dinv0[inv]

    # self loops as regular edges
    loop = np.arange(N, dtype=np.int64)
    src = np.concatenate([src, loop])
    dst = np.concatenate([dst, loop])

    order = np.argsort(dst, kind="stable")
    s_s = src[order]
    d_s = dst[order]
    bounds = np.searchsorted(d_s, np.arange(NC + 1) * SH)

    xs = (dinv[:, None] * x).astype(BF16)  # [N, 128] pre-scaled features

    # source piece + piece-local table row for every edge
    gc = s_s // SH
    gr = s_s % SH
    q_of = np.searchsorted(np.array(AG_LO[1:]), gr, side="right")  # [TE]
    rq = np.array(RQ)[q_of]
    lo = np.array(AG_LO)[q_of]
    prow = gc * rq + (gr - lo)         # row within piece-q gathered table
    blk = np.minimum((d_s % SH) >> 7, NBLK - 1)

    # per-core per-(block, piece) counts -> shared chunk counts k2
    cnt = np.zeros((NC, NBLK, NQ), dtype=np.int64)
    for c in range(NC):
        lo_, hi_ = bounds[c], bounds[c + 1]
        np.add.at(cnt[c], (blk[lo_:hi_], q_of[lo_:hi_]), 1)
    k2 = (cnt.max(axis=0) + P - 1) // P            # [NBLK, NQ]
    cnt1 = cnt.sum(axis=2)                         # [NC, NBLK]
    k1 = np.maximum(1, (cnt1.max(axis=0) + P - 1) // P)  # [NBLK]
    cum1 = np.concatenate([[0], np.cumsum(k1)]).astype(int)
    nch1 = int(cum1[-1])

    # L2/L3 column layout: group-major, then piece, then block
    colstart = np.zeros((NBLK, NQ), dtype=np.int64)
    cstart = np.zeros((NG, NQ), dtype=np.int64)
    K = np.zeros((NG, NQ), dtype=np.int64)
    gstart = np.zeros(NG + 1, dtype=np.int64)
    pos = 0
    for g in range(NG):
        gstart[g] = pos
        for q in range(NQ):
            cstart[g, q] = pos
            for b in range(g * GSZ, (g + 1) * GSZ):
                colstart[b, q] = pos
                pos += k2[b, q]
            K[g, q] = pos - cstart[g, q]
    gstart[NG] = pos
    nch2 = int(pos)
    cols2 = [
        [int(colstart[b, q]) + i for q in range(NQ) for i in range(k2[b, q])]
        for b in range(NBLK)
    ]
    # block-major S-table column layout (for contiguous per-block S build)
    cp2 = k2.sum(axis=1)                        # chunks per block
    scum2 = np.concatenate([[0], np.cumsum(cp2)]).astype(int)
    # global gt column -> block-major S column
    s_of_col = np.zeros(nch2, dtype=np.int64)
    for b in range(NBLK):
        for ci, col in enumerate(cols2[b]):
            s_of_col[col] = scum2[b] + ci
    wstart = np.zeros((NG, NQ), dtype=np.int64)
    tw = 0
    for g in range(NG):
        for q in range(NQ):
            wstart[g, q] = tw
            tw += 8 * int(K[g, q])
    TW = int(tw)

    per_core = []
    for c in range(NC):
        lo_, hi_ = bounds[c], bounds[c + 1]
        b_c = blk[lo_:hi_]
        q_c = q_of[lo_:hi_]
        pr_c = prow[lo_:hi_]
        d_c = d_s[lo_:hi_] - c * SH
        s_c = s_s[lo_:hi_]

        # ---- L2/L3 slot assignment: order by (block, piece) ----
        key = b_c * NQ + q_c
        o2 = np.argsort(key, kind="stable")
        key_s = key[o2]
        # position within each (b,q) run
        starts = np.searchsorted(key_s, np.arange(NBLK * NQ))
        pos_in = np.arange(len(key_s)) - starts[key_s]
        chunk = pos_in >> 7
        part = pos_in & 127
        col = colstart.reshape(-1)[key_s] + chunk     # global chunk col

        scol = s_of_col[col]                    # block-major S column
        dstcol2 = np.full((P, nch2), -1, dtype=np.int16)
        dinvd2 = np.zeros((P, nch2), dtype=np.float32)  # cast to bf16 below
        dloc = d_c[o2]
        dstcol2[part, scol] = (dloc & 127).astype(np.int16)
        dinvd2[part, scol] = dinv[dloc + c * SH]

        # gather index table: wrapped-16 i16 layout per (g, q) region
        big = np.zeros(P * nch2, dtype=np.int16)
        big[col * P + part] = pr_c[o2].astype(np.int16)
        idx2 = np.zeros((P, TW), dtype=np.int16)
        for g in range(NG):
            for q in range(NQ):
                kk = int(K[g, q])
                if kk == 0:
                    continue
                arr = big[int(cstart[g, q]) * P:(int(cstart[g, q]) + kk) * P]
                wr = arr.reshape(-1, 16).T            # [16, 8*kk]
                ws = int(wstart[g, q])
                idx2[:, ws:ws + 8 * kk] = np.tile(wr, (8, 1))

        # ---- L1 slot assignment: order by block only ----
        o1 = np.argsort(b_c, kind="stable")
        b1s = b_c[o1]
        starts1 = np.searchsorted(b1s, np.arange(NBLK))
        pos1 = np.arange(len(b1s)) - starts1[b1s]
        chunk_1 = pos1 >> 7
        part1 = pos1 & 127
        col1 = cum1[b1s] + chunk_1

        xe = np.zeros((P, nch1, DIN), dtype=BF16)
        xe[part1, col1, :] = xs[s_c[o1]]
        dstcol1 = np.full((P, nch1), -1, dtype=np.int16)
        dinvd1 = np.zeros((P, nch1), dtype=np.float32)
        d1 = d_c[o1]
        dstcol1[part1, col1] = (d1 & 127).astype(np.int16)
        dinvd1[part1, col1] = dinv[d1 + c * SH]
        xe = xe.reshape(P, nch1 * DIN)

        # dinv of own shard in [p, w] window layout (phase-1 scaling)
        ids = c * SH + np.arange(NBLK * P)
        valid = ids < (c + 1) * SH
        dc_own = np.where(valid, dinv[np.minimum(ids, N - 1)], 0.0)
        dc_own = dc_own.reshape(NBLK, P).T.astype(np.float32).copy()

        per_core.append({
            "xe": xe,
            "dc1": dstcol1, "dv1": dinvd1.astype(BF16),
            "dc2": dstcol2, "dv2": dinvd2.astype(BF16),
            "idx2": idx2,
            "dco": dc_own,
        })

    meta = {
        "k2": k2.tolist(), "k1": k1.tolist(), "cum1": cum1.tolist(),
        "nch1": nch1, "nch2": nch2, "TW": TW,
        "cstart": cstart.tolist(), "K": K.tolist(),
        "gstart": gstart.tolist(), "wstart": wstart.tolist(),
        "cols2": cols2, "cp2": cp2.tolist(), "scum2": scum2.tolist(),
    }
    return per_core, meta, perm


def _build_program(meta):
    from concourse import bass, bacc, mybir
    import concourse.tile as tile

    f32 = mybir.dt.float32
    bf16 = mybir.dt.bfloat16
    i16 = mybir.dt.int16
    i32 = mybir.dt.int32
    k1 = meta["k1"]
    cum1 = meta["cum1"]
    nch1 = meta["nch1"]
    nch2 = meta["nch2"]
    TW = meta["TW"]
    wstart = meta["wstart"]
    cstart = meta["cstart"]
    K = meta["K"]
    gstart = meta["gstart"]
    cols2 = meta["cols2"]
    cp2 = meta["cp2"]
    scum2 = meta["scum2"]
    mxk1 = max(k1)
    mxcp = max(max(k1), max(cp2))
    mxgw = max(gstart[g + 1] - gstart[g] for g in range(NG))

    nc = bacc.Bacc("TRN2", target_bir_lowering=False, debug=False,
                   dynamic_dma_scratch_size=65536)

    xe = nc.declare_dram_parameter("xe", [P, nch1 * DIN], bf16, isOutput=False)
    dc1 = nc.declare_dram_parameter("dc1", [P, nch1], i16, isOutput=False)
    dv1 = nc.declare_dram_parameter("dv1", [P, nch1], bf16, isOutput=False)
    dc2 = nc.declare_dram_parameter("dc2", [P, nch2], i16, isOutput=False)
    dv2 = nc.declare_dram_parameter("dv2", [P, nch2], bf16, isOutput=False)
    idx2 = nc.declare_dram_parameter("idx2", [P, TW], i16, isOutput=False)
    dco = nc.declare_dram_parameter("dco", [P, NBLK], f32, isOutput=False)
    w1 = nc.declare_dram_parameter("w1", [P, DH], bf16, isOutput=False)
    w2p = nc.declare_dram_parameter("w2p", [P, 2 * DH], bf16, isOutput=False)
    w3p = nc.declare_dram_parameter("w3p", [P, 2 * DH], bf16, isOutput=False)
    bt = nc.declare_dram_parameter("bt", [P, 4], f32, isOutput=False)
    bf3 = nc.declare_dram_parameter("bf3", [P, DH], f32, isOutput=False)
    iot = nc.declare_dram_parameter("iot", [P, mxcp * P], i16, isOutput=False)
    outp = nc.declare_dram_parameter("out", [SH, DH], f32, isOutput=True)

    ybin2 = [nc.dram_tensor(f"ybin2_{q}", [RQ[q], DH], bf16) for q in range(NQ)]
    ybout2 = [nc.dram_tensor(f"ybout2_{q}", [NC * RQ[q], DH], bf16,
                             addr_space="Shared") for q in range(NQ)]
    ybin3 = [nc.dram_tensor(f"ybin3_{q}", [RQ[q], DH], bf16) for q in range(NQ)]
    ybout3 = [nc.dram_tensor(f"ybout3_{q}", [NC * RQ[q], DH], bf16,
                             addr_space="Shared") for q in range(NQ)]

    AG = mybir.AluOpType
    ACT = mybir.ActivationFunctionType

    def piece_of_window(w):
        for q in range(NQ):
            if AG_WB[q] <= w < AG_WB[q + 1]:
                return q
        raise AssertionError

    with tile.TileContext(nc, linearize=bool(os.environ.get("KLIN"))) as tc:
        with (
            tc.tile_pool(name="const", bufs=1) as cp_,
            tc.tile_pool(name="sb", bufs=2) as sb,
            tc.tile_pool(name="stp", bufs=2) as stp,
            tc.tile_pool(name="gp", bufs=2) as gp,
            tc.tile_pool(name="xb", bufs=2) as xbp,
            tc.tile_pool(name="pp", bufs=2, space="PSUM") as pp,
            tc.tile_pool(name="ph", bufs=6, space="PSUM") as ph,
        ):
            w1sb = cp_.tile([P, DH], dtype=bf16)
            nc.sync.dma_start(out=w1sb[:], in_=w1[:, :])
            w2sb = cp_.tile([P, 2 * DH], dtype=bf16)
            nc.sync.dma_start(out=w2sb[:], in_=w2p[:, :])
            w3sb = cp_.tile([P, 2 * DH], dtype=bf16)
            nc.sync.dma_start(out=w3sb[:], in_=w3p[:, :])
            btsb = cp_.tile([P, 4], dtype=f32)
            nc.sync.dma_start(out=btsb[:], in_=bt[:, :])
            bf3sb = cp_.tile([P, DH], dtype=f32)
            nc.sync.dma_start(out=bf3sb[:], in_=bf3[:, :])
            dcosb = cp_.tile([P, NBLK], dtype=f32)
            nc.sync.dma_start(out=dcosb[:], in_=dco[:, :])
            iotsb = cp_.tile([P, mxcp * P], dtype=i16)
            nc.sync.dma_start(out=iotsb[:], in_=iot[:, :])
            dc1sb = cp_.tile([P, nch1], dtype=i16)
            nc.sync.dma_start(out=dc1sb[:], in_=dc1[:, :])
            dv1sb = cp_.tile([P, nch1], dtype=bf16)
            nc.sync.dma_start(out=dv1sb[:], in_=dv1[:, :])
            dc2sb = cp_.tile([P, nch2], dtype=i16)
            nc.sync.dma_start(out=dc2sb[:], in_=dc2[:, :])
            dv2sb = cp_.tile([P, nch2], dtype=bf16)
            nc.sync.dma_start(out=dv2sb[:], in_=dv2[:, :])
            idxsb = cp_.tile([P, TW], dtype=i16)
            nc.sync.dma_start(out=idxsb[:], in_=idx2[:, :])
            # resident transposed activations h^T: half h at cols [h*SH, ...)
            xts = cp_.tile([P, 2 * SH], dtype=bf16)

            def sbuild_block(dcsb, dvsb, c0, cp):
                """All of a block's one-hot scatter chunks in two DVE ops:
                S[p, k*128+c] = (c == dstcol[p, c0+k]) * dinv[p, c0+k]."""
                eq = stp.tile([P, mxcp * P], dtype=bf16, tag="eq")
                nc.vector.tensor_tensor(
                    out=eq[:, :cp * P].rearrange("p (k e) -> p k e", e=P),
                    in0=iotsb[:, :cp * P].rearrange("p (k e) -> p k e", e=P),
                    in1=dcsb[:, c0:c0 + cp].to_broadcast((P, cp, P)),
                    op=AG.is_equal)
                st = stp.tile([P, mxcp * P], dtype=bf16, tag="st")
                nc.vector.tensor_tensor(
                    out=st[:, :cp * P].rearrange("p (k e) -> p k e", e=P),
                    in0=eq[:, :cp * P].rearrange("p (k e) -> p k e", e=P),
                    in1=dvsb[:, c0:c0 + cp].to_broadcast((P, cp, P)),
                    op=AG.mult)
                return st

            def phase1_win(wsb, ybinq, w):
                """One window of Y = dinv * (h @ W) from xts -> ybin rows."""
                m = LASTM if w == NBLK - 1 else P
                ps = pp.tile([P, DH], dtype=f32, tag="ps")
                for h in range(2):
                    nc.tensor.matmul(
                        out=ps[:m, :],
                        lhsT=xts[:, h * SH + w * P:h * SH + w * P + m],
                        rhs=wsb[:, h * DH:(h + 1) * DH],
                        start=(h == 0), stop=(h == 1))
                ysb = sb.tile([P, DH], dtype=bf16, tag="ysb")
                nc.scalar.activation(out=ysb[:m, :], in_=ps[:m, :],
                                     func=ACT.Copy,
                                     scale=dcosb[:m, w:w + 1])
                q = piece_of_window(w)
                r0 = w * P - AG_LO[q]
                nc.sync.dma_start(out=ybinq[q][r0:r0 + m, :], in_=ysb[:m, :])

            def all_gather_piece(ybinq, yboutq, q):
                nc.gpsimd.collective_compute(
                    "AllGather", AG.bypass,
                    replica_groups=[list(range(NC))],
                    ins=[ybinq[q][0:RQ[q], :].opt()],
                    outs=[yboutq[q][0:NC * RQ[q], :].opt()])

            gmax = int(os.environ.get("KGMAX", "8"))   # chunks per gather

            def group_gather(g, tableq):
                """Batched gathers for group g: <=gmax-chunk dma_gathers per
                piece (the wrapped-16 idx layout slices at chunk granularity)."""
                gt = gp.tile([P, mxgw * DH], dtype=bf16, tag="gt")
                g0 = gstart[g]
                for q in range(NQ):
                    kk = K[g][q]
                    c0 = cstart[g][q] - g0      # column offset inside tile
                    ws = wstart[g][q]
                    for j0 in range(0, kk, gmax):
                        j1 = min(j0 + gmax, kk)
                        out_ap = gt[:, (c0 + j0) * DH:(c0 + j1) * DH].rearrange(
                            "p (k e) -> p k e", e=DH)
                        nc.gpsimd.dma_gather(
                            out_ap,
                            tableq[q][0:NC * RQ[q], :],
                            idxsb[:, ws + 8 * j0:ws + 8 * j1],
                            P * (j1 - j0),
                            P * (j1 - j0),
                            DH)
                return gt

            # ---------------- Layer 1: streamed edge table ------------------
            for b in range(NBLK):
                kb = k1[b]
                m = LASTM if b == NBLK - 1 else P
                xet = xbp.tile([P, mxk1 * DIN], dtype=bf16, tag="xet")
                nc.sync.dma_start(
                    out=xet[:, :kb * DIN],
                    in_=xe[:, cum1[b] * DIN:(cum1[b] + kb) * DIN])
                psa = ph.tile([P, P], dtype=f32, tag="half")
                stb = sbuild_block(dc1sb, dv1sb, cum1[b], kb)
                for i in range(kb):
                    nc.tensor.matmul(
                        out=psa[:, :m],
                        lhsT=xet[:, i * DIN:(i + 1) * DIN],
                        rhs=stb[:, i * P:i * P + m],
                        start=(i == 0), stop=(i == kb - 1))
                agg = sb.tile([P, P], dtype=bf16, tag="agg")
                nc.scalar.activation(out=agg[:, :m], in_=psa[:, :m],
                                     func=ACT.Copy)
                psb = [ph.tile([P, P], dtype=f32, tag="half", name=f"psb{h}")
                       for h in range(2)]
                for h in range(2):
                    nc.tensor.matmul(
                        out=psb[h][:, :m],
                        lhsT=w1sb[:, h * P:(h + 1) * P],
                        rhs=agg[:, :m],
                        start=True, stop=True)
                for h in range(2):
                    nc.scalar.activation(
                        out=xts[:, h * SH + b * P:h * SH + b * P + m],
                        in_=psb[h][:, :m],
                        func=ACT.Relu, bias=btsb[:, h:h + 1])
                phase1_win(w2sb, ybin2, b)
                if b + 1 in AG_WB[1:4]:
                    all_gather_piece(ybin2, ybout2, AG_WB.index(b + 1) - 1)
            all_gather_piece(ybin2, ybout2, 3)

            # ---------------- Layer 2: transposed scatter -------------------
            for g in range(NG):
                gt = group_gather(g, ybout2)
                g0 = gstart[g]
                for b in range(g * GSZ, (g + 1) * GSZ):
                    m = LASTM if b == NBLK - 1 else P
                    cols = cols2[b]
                    pst = [ph.tile([P, P], dtype=f32, tag="half",
                                   name=f"pst{h}") for h in range(2)]
                    stb = sbuild_block(dc2sb, dv2sb, scum2[b], len(cols))
                    for ci, col in enumerate(cols):
                        lp = col - g0
                        for h in range(2):
                            nc.tensor.matmul(
                                out=pst[h][:, :m],
                                lhsT=gt[:, lp * DH + h * P:lp * DH + (h + 1) * P],
                                rhs=stb[:, ci * P:ci * P + m],
                                start=(ci == 0), stop=(ci == len(cols) - 1))
                    for h in range(2):
                        nc.scalar.activation(
                            out=xts[:, h * SH + b * P:h * SH + b * P + m],
                            in_=pst[h][:, :m],
                            func=ACT.Relu, bias=btsb[:, 2 + h:2 + h + 1])
                    phase1_win(w3sb, ybin3, b)
                    if b + 1 in AG_WB[1:4]:
                        all_gather_piece(ybin3, ybout3, AG_WB.index(b + 1) - 1)
            all_gather_piece(ybin3, ybout3, 3)

            # ---------------- Layer 3: direct scatter -> out ----------------
            for g in range(NG):
                gt = group_gather(g, ybout3)
                g0 = gstart[g]
                for b in range(g * GSZ, (g + 1) * GSZ):
                    m = LASTM if b == NBLK - 1 else P
                    cols = cols2[b]
                    ps3 = pp.tile([P, DH], dtype=f32, tag="ps")
                    stb = sbuild_block(dc2sb, dv2sb, scum2[b], len(cols))
                    for ci, col in enumerate(cols):
                        lp = col - g0
                        nc.tensor.matmul(
                            out=ps3[:m, :],
                            lhsT=stb[:, ci * P:ci * P + m],
                            rhs=gt[:, lp * DH:(lp + 1) * DH],
                            start=(ci == 0), stop=(ci == len(cols) - 1))
                    osb = sb.tile([P, DH], dtype=f32, tag="osb")
                    nc.vector.tensor_tensor(out=osb[:m, :], in0=ps3[:m, :],
                                            in1=bf3sb[:m, :], op=AG.add)
                    nc.sync.dma_start(out=outp[b * P:b * P + m, :],
                                      in_=osb[:m, :])

    nc.compile()
    return nc


def kernel(x, edge_index, W1, b1, W2, b2, W3, b3, _trace=False):
    from concourse.bass_utils import run_bass_kernel_spmd

    x = np.asarray(x, dtype=np.float32)
    per_core, meta, perm = _preprocess(x, edge_index)
    nc = _build_program(meta)

    w2 = np.asarray(W2, np.float32)
    w3 = np.asarray(W3, np.float32)
    w2p = np.concatenate([w2[0:P, :], w2[P:2 * P, :]], axis=1).astype(BF16)
    w3p = np.concatenate([w3[0:P, :], w3[P:2 * P, :]], axis=1).astype(BF16)
    b1v = np.asarray(b1, np.float32)
    b2v = np.asarray(b2, np.float32)
    bt = np.stack([b1v[0:P], b1v[P:2 * P], b2v[0:P], b2v[P:2 * P]],
                  axis=1).astype(np.float32)
    common = {
        "w1": np.asarray(W1, np.float32).astype(BF16),
        "w2p": w2p,
        "w3p": w3p,
        "bt": bt,
        "bf3": np.broadcast_to(np.asarray(b3, np.float32), (P, DH)).copy(),
    }
    mxcp = max(max(meta["k1"]), max(meta["cp2"]))
    common["iot"] = np.broadcast_to(
        np.tile(np.arange(P, dtype=np.int16), mxcp), (P, mxcp * P)).copy()
    in_maps = []
    for c in range(NC):
        m = dict(common)
        m.update(per_core[c])
        in_maps.append(m)

    res = run_bass_kernel_spmd(nc, in_maps, list(range(NC)), trace=_trace)
    shards = [res.results[c]["out"] for c in range(NC)]
    out = np.concatenate(shards, axis=0)[perm]
    if _trace:
        return out, res
    return out
